# revision 3
# baseline (speedup 1.0000x reference)
"""Trainium2 Bass kernel for the DataReloadingQNN problem.

Math: layers 0..4 plus the shared RZ/RY/RZ of layer 5 collapse into one
fixed state w (params only).  The data gates are RY(x_q) = c_q I + s_q J_q
with J a signed permutation, all commuting.  Peel the three LSB qubits
(q=8,9,10): contract the other eight via a dense matmul
    T[b, :] = W_lo[b, :] @ U,   W_lo = tensor product of 8 [cos,sin] pairs,
    U[m, :] = (P J^{(m)} w) re/im-interleaved, m in [0,256), P = CNOT chain,
then apply, per peeled qubit q, the per-sample signed-permutation rotation
    T <- c_q T + s_q * sign_q(col) * T[col ^ M_q]
with (M, period) = (14,32), (6,16), (2,8) in interleaved column space and
sign +1 on the middle half of each period (verified vs reference in f64).

Device work per core (1024 samples = 8 tiles of 128):
  - cos/sin/-sin of x/2 on ScalarE
  - W_lo (128 x 256 per tile) by doubling on VectorE; PE-transpose
  - matmul K=256 into PSUM per 512-col chunk; ScalarE copies PSUM->SBUF bf16
  - 3 rotations: dense scaled copy (p1) + 2-4 strided STT sub-calls (p2)
  - DMA out bf16; host upcasts to f32
Inputs sharded batch-wise across 8 cores; U replicated.
"""
import numpy as np
import ml_dtypes

import concourse.bass as bass
import concourse.bacc as bacc
import concourse.tile as tile
from concourse import mybir
from concourse.bass_utils import run_bass_kernel_spmd

N = 11
DIM = 2048
BATCH = 8192
NCORES = 8
BSH = BATCH // NCORES          # 1024 samples per core
NTILES = BSH // 128            # 8 sample-tiles per core
KLO = 8                        # qubits contracted in the matmul
NU = 1 << KLO                  # 256 = rows of U
PEEL = (8, 9, 10)              # peeled qubits, rotation order
W2 = 2 * DIM                   # 4096 output columns (re/im interleaved)
NCH = 8                        # 512-col chunks
CW = W2 // NCH
F32 = mybir.dt.float32
BF16 = mybir.dt.bfloat16

MUL = mybir.AluOpType.mult
ADD = mybir.AluOpType.add

# ---------------------------------------------------------------- host math


def _rz(phi):
    e = np.exp(-0.5j * phi)
    return np.array([[e, 0], [0, np.conj(e)]], dtype=np.complex128)


def _ry(theta):
    t = 0.5 * theta
    c, s = np.cos(t), np.sin(t)
    return np.array([[c, -s], [s, c]], dtype=np.complex128)


def _apply_1q_rows(rows, U, q):
    R = rows.shape[0]
    st = rows.reshape(R, 2 ** q, 2, 2 ** (N - 1 - q))
    st = np.einsum('ab,rxby->rxay', U, st)
    return st.reshape(R, DIM)


def _apply_cnot_rows(rows, c):
    R = rows.shape[0]
    st = rows.reshape(R, 2 ** c, 2, 2, 2 ** (N - 2 - c))
    st = np.stack([st[:, :, 0], st[:, :, 1, ::-1]], axis=2)
    return st.reshape(R, DIM)


def build_u_matrix(params):
    """(6,11,3) f32 -> U (256, 4096) f64: rows = P J^{(m)} w over subsets m
    of qubits 0..7, re/im interleaved columns, CNOT permutation folded."""
    p = params.astype(np.float64)
    v = np.zeros((1, DIM), dtype=np.complex128)
    v[0, 0] = 1.0
    for l in range(5):
        for q in range(N):
            v = _apply_1q_rows(v, _rz(p[l, q, 0]), q)
            v = _apply_1q_rows(v, _ry(p[l, q, 1]), q)
            v = _apply_1q_rows(v, _rz(p[l, q, 2]), q)
        for c in range(N - 1):
            v = _apply_cnot_rows(v, c)
    for q in range(N):
        B = _rz(p[5, q, 2]) @ _ry(p[5, q, 1]) @ _rz(p[5, q, 0])
        v = _apply_1q_rows(v, B, q)

    # rows over J-subsets of qubits 0..7 (bit b of m <-> qubit b)
    rows = v                    # (1, 2048)
    idx = np.arange(DIM)
    for q in range(KLO):
        m = 1 << (N - 1 - q)
        sgn = np.where(idx & m, 1.0, -1.0)
        rows = np.concatenate([rows, sgn * rows[:, idx ^ m]], axis=0)

    # fold CNOT-chain permutation: final[j] = pre[g[j]]
    g = np.arange(DIM)[None, :]
    for c in range(N - 1):
        g = _apply_cnot_rows(g.astype(np.float64), c).astype(np.int64)
    rows = rows[:, g[0]]

    U = np.empty((NU, W2), dtype=np.float64)
    U[:, 0::2] = rows.real
    U[:, 1::2] = rows.imag
    return U


# ------------------------------------------------------------- bass kernel


def _rot_p2(eng, dst, src, u, s_pos, s_neg, period):
    """dst = (src[col ^ M] * (+-s)) + u, M = period/2 - 2, sign +1 on the
    middle half of each period.  All tiles [128, 4096] bf16.

    Walrus caps TensorScalarPtr APs at 3 dims (p + 2 free), so emit one
    sub-call per re/im pair-offset: out cols {P*g + oc, +1} read
    {P*g + oc^M, +1}."""
    Q = period // 4
    M = period // 2 - 2
    vd = dst[:].rearrange("p (g u) -> p g u", u=period)
    vs = src[:].rearrange("p (g u) -> p g u", u=period)
    vu = u[:].rearrange("p (g u) -> p g u", u=period)
    for ho, s in ((1, s_pos), (2, s_pos), (0, s_neg), (3, s_neg)):
        for b in range(Q // 2):
            oc = ho * Q + 2 * b
            ic = oc ^ M
            eng.scalar_tensor_tensor(vd[:, :, oc:oc + 2],
                                     vs[:, :, ic:ic + 2], s,
                                     vu[:, :, oc:oc + 2], MUL, ADD)


def build_kernel():
    nc = bacc.Bacc()
    x_d = nc.dram_tensor("x", (BSH, N), F32, kind="ExternalInput")
    u_d = nc.dram_tensor("u", (2, 128, W2), BF16, kind="ExternalInput")
    id_d = nc.dram_tensor("ident", (128, 128), BF16, kind="ExternalInput")
    out_d = nc.dram_tensor("out", (BSH, W2), BF16, kind="ExternalOutput")

    with tile.TileContext(nc) as tc:
        with (
            tc.tile_pool(name="const", bufs=1) as const_pool,
            tc.tile_pool(name="wbuild", bufs=2) as wbuild_pool,
            tc.tile_pool(name="wt", bufs=1) as wt_pool,
            tc.tile_pool(name="rot", bufs=2) as rot_pool,
            tc.tile_pool(name="ptr", bufs=2, space=bass.MemorySpace.PSUM) as ptr_pool,
            tc.tile_pool(name="pmm", bufs=4, space=bass.MemorySpace.PSUM) as pmm_pool,
        ):
            ident = const_pool.tile([128, 128], BF16)
            nc.gpsimd.dma_start(ident[:], id_d[:])

            # U replicated: 2 k-chunks of [128, 4096]
            u_sb = []
            for k in range(2):
                ut = const_pool.tile([128, W2], BF16, tag=f"u{k}")
                nc.sync.dma_start(ut[:], u_d[k])
                u_sb.append(ut)

            # x: (1024, 11) -> sbuf (128, 8*11); tile t in cols [t*11,(t+1)*11)
            x_sb = const_pool.tile([128, NTILES * N], F32)
            x_r = x_d.rearrange("(t p) f -> p t f", p=128)
            nc.gpsimd.dma_start(x_sb[:].rearrange("p (t f) -> p t f", f=N), x_r)

            cos_sb = const_pool.tile([128, NTILES * N], F32)
            sin_sb = const_pool.tile([128, NTILES * N], F32)
            nsin_sb = const_pool.tile([128, NTILES * N], F32)
            hp_t = const_pool.tile([128, 1], F32)
            zr_t = const_pool.tile([128, 1], F32)
            nc.vector.memset(hp_t[:], float(np.pi / 2))
            nc.vector.memset(zr_t[:], 0.0)
            # cos(t) = sin(pi/2 - t): keeps Sin args in (-pi/2, pi/2]
            nc.scalar.activation(cos_sb[:], x_sb[:],
                                 mybir.ActivationFunctionType.Sin,
                                 bias=hp_t[:], scale=-0.5)
            nc.scalar.activation(sin_sb[:], x_sb[:],
                                 mybir.ActivationFunctionType.Sin,
                                 bias=zr_t[:], scale=0.5)
            nc.scalar.activation(nsin_sb[:], x_sb[:],
                                 mybir.ActivationFunctionType.Sin,
                                 bias=zr_t[:], scale=-0.5)

            # Phase A: build transposed W_lo for every sample-tile
            wts = []
            for t in range(NTILES):
                col = t * N
                wa = wbuild_pool.tile([128, NU], F32, tag="wa")
                wb = wbuild_pool.tile([128, NU], F32, tag="wb")
                nc.vector.tensor_copy(wa[:, 0:1], cos_sb[:, col:col + 1])
                nc.vector.tensor_copy(wa[:, 1:2], sin_sb[:, col:col + 1])
                cur, nxt = wa, wb
                for j in range(1, KLO):
                    half = 1 << j
                    nc.vector.tensor_scalar_mul(
                        nxt[:, 0:half], cur[:, 0:half],
                        cos_sb[:, col + j:col + j + 1])
                    nc.vector.tensor_scalar_mul(
                        nxt[:, half:2 * half], cur[:, 0:half],
                        sin_sb[:, col + j:col + j + 1])
                    cur, nxt = nxt, cur
                wbf = wbuild_pool.tile([128, NU], BF16, tag="wbf")
                nc.vector.tensor_copy(wbf[:], cur[:])

                wt = wt_pool.tile([128, NU], BF16, tag=f"wt{t}")
                for k in range(2):
                    ptr = ptr_pool.tile([128, 128], BF16)
                    nc.tensor.transpose(ptr[:], wbf[:, k * 128:(k + 1) * 128],
                                        ident[:])
                    nc.vector.tensor_copy(wt[:, k * 128:(k + 1) * 128], ptr[:])
                wts.append(wt)

            # Phase B: matmul + rotations per sample-tile
            for t in range(NTILES):
                col = t * N
                S = rot_pool.tile([128, W2], BF16, tag="S")
                for ci in range(NCH):
                    pmm = pmm_pool.tile([128, CW], F32)
                    nc.tensor.matmul(pmm[:], wts[t][:, 0:128],
                                     u_sb[0][:, ci * CW:(ci + 1) * CW],
                                     start=True, stop=False)
                    nc.tensor.matmul(pmm[:], wts[t][:, 128:256],
                                     u_sb[1][:, ci * CW:(ci + 1) * CW],
                                     start=False, stop=True)
                    nc.scalar.copy(S[:, ci * CW:(ci + 1) * CW], pmm[:])

                V = rot_pool.tile([128, W2], BF16, tag="V")
                T1 = rot_pool.tile([128, W2], BF16, tag="T1")
                T2 = rot_pool.tile([128, W2], BF16, tag="T2")
                T3 = rot_pool.tile([128, W2], BF16, tag="T3")

                def csn(q):
                    return (cos_sb[:, col + q:col + q + 1],
                            sin_sb[:, col + q:col + q + 1],
                            nsin_sb[:, col + q:col + q + 1])

                c8, s8, n8 = csn(8)
                c9, s9, n9 = csn(9)
                c10, s10, n10 = csn(10)

                # r8: period 32
                nc.scalar.mul(V[:], S[:], c8)
                _rot_p2(nc.vector, T1, S, V, s8, n8, 32)
                # r9: period 16
                nc.vector.tensor_scalar_mul(V[:], T1[:], c9)
                _rot_p2(nc.vector, T2, T1, V, s9, n9, 16)
                # r10: period 8
                nc.gpsimd.tensor_scalar_mul(V[:], T2[:], c10)
                _rot_p2(nc.vector, T3, T2, V, s10, n10, 8)

                nc.sync.dma_start(out_d[t * 128:(t + 1) * 128, :], T3[:])
    nc.finalize()
    return nc


# ----------------------------------------------------------------- driver

_CACHE = {}


def kernel(X, params):
    X = np.ascontiguousarray(np.asarray(X, dtype=np.float32))
    params = np.asarray(params, dtype=np.float32)

    U = build_u_matrix(params)
    u_bf = np.ascontiguousarray(
        U.reshape(2, 128, W2).astype(ml_dtypes.bfloat16))
    ident = np.eye(128, dtype=ml_dtypes.bfloat16)

    if "nc" not in _CACHE:
        _CACHE["nc"] = build_kernel()
    nc = _CACHE["nc"]

    in_maps = []
    for c in range(NCORES):
        in_maps.append({
            "x": X[c * BSH:(c + 1) * BSH],
            "u": u_bf,
            "ident": ident,
        })
    res = run_bass_kernel_spmd(nc, in_maps, list(range(NCORES)))
    out = np.concatenate([res.results[c]["out"] for c in range(NCORES)],
                         axis=0)
    return out.astype(np.float32).reshape(BATCH, DIM, 2)


# revision 7
# speedup vs baseline: 3.7342x; 3.7342x over previous
"""Trainium2 Bass kernel for the DataReloadingQNN problem.

Math: layers 0..4 plus the shared RZ/RY/RZ of layer 5 collapse into one
fixed state w (params only).  The data gates are RY(x_q) = c_q I + s_q J_q
with J a signed permutation, all commuting.  Peel the three LSB qubits
(q=8,9,10): contract the other eight via a dense matmul
    T[b, :] = W_lo[b, :] @ U,   W_lo = tensor product of 8 [cos,sin] pairs,
    U[m, :] = (P J^{(m)} w) re/im-interleaved, m in [0,256), P = CNOT chain,
then apply, per peeled qubit q, the per-sample signed-permutation rotation
    T <- c_q T + s_q * sign_q(col) * T[col ^ M_q]
with (M, period) = (14,32), (6,16), (2,8) in interleaved column space and
sign +1 on the middle half of each period (verified vs reference in f64).

Device work per core (1024 samples = 8 tiles of 128):
  - cos/sin/-sin of x/2 on ScalarE
  - W_lo (128 x 256 per tile) by doubling on VectorE; PE-transpose
  - matmul K=256 into PSUM per 512-col chunk; ScalarE copies PSUM->SBUF bf16
  - 3 rotations: dense scaled copy (p1) + 2-4 strided STT sub-calls (p2)
  - DMA out bf16; host upcasts to f32
Inputs sharded batch-wise across 8 cores; U replicated.
"""
import numpy as np
import ml_dtypes

import concourse.bass as bass
import concourse.bacc as bacc
import concourse.tile as tile
from concourse import mybir
from concourse.bass_utils import run_bass_kernel_spmd

N = 11
DIM = 2048
BATCH = 8192
NCORES = 8
BSH = BATCH // NCORES          # 1024 samples per core
NTILES = BSH // 128            # 8 sample-tiles per core
KLO = 8                        # qubits contracted in the matmul
NU = 1 << KLO                  # 256 = rows of U
PEEL = (8, 9, 10)              # peeled qubits, rotation order
W2 = 2 * DIM                   # 4096 output columns (re/im interleaved)
NCH = 8                        # 512-col chunks
CW = W2 // NCH
F32 = mybir.dt.float32
BF16 = mybir.dt.bfloat16

MUL = mybir.AluOpType.mult
ADD = mybir.AluOpType.add

# ---------------------------------------------------------------- host math


def _rz(phi):
    e = np.exp(-0.5j * phi)
    return np.array([[e, 0], [0, np.conj(e)]], dtype=np.complex128)


def _ry(theta):
    t = 0.5 * theta
    c, s = np.cos(t), np.sin(t)
    return np.array([[c, -s], [s, c]], dtype=np.complex128)


def _apply_1q_rows(rows, U, q):
    R = rows.shape[0]
    st = rows.reshape(R, 2 ** q, 2, 2 ** (N - 1 - q))
    st = np.einsum('ab,rxby->rxay', U, st)
    return st.reshape(R, DIM)


def _apply_cnot_rows(rows, c):
    R = rows.shape[0]
    st = rows.reshape(R, 2 ** c, 2, 2, 2 ** (N - 2 - c))
    st = np.stack([st[:, :, 0], st[:, :, 1, ::-1]], axis=2)
    return st.reshape(R, DIM)


def build_u_matrix(params):
    """(6,11,3) f32 -> U (256, 4096) f64: rows = P J^{(m)} w over subsets m
    of qubits 0..7, re/im interleaved columns, CNOT permutation folded."""
    p = params.astype(np.float64)
    v = np.zeros((1, DIM), dtype=np.complex128)
    v[0, 0] = 1.0
    for l in range(5):
        for q in range(N):
            v = _apply_1q_rows(v, _rz(p[l, q, 0]), q)
            v = _apply_1q_rows(v, _ry(p[l, q, 1]), q)
            v = _apply_1q_rows(v, _rz(p[l, q, 2]), q)
        for c in range(N - 1):
            v = _apply_cnot_rows(v, c)
    for q in range(N):
        B = _rz(p[5, q, 2]) @ _ry(p[5, q, 1]) @ _rz(p[5, q, 0])
        v = _apply_1q_rows(v, B, q)

    # rows over J-subsets of qubits 0..7 (bit b of m <-> qubit b)
    rows = v                    # (1, 2048)
    idx = np.arange(DIM)
    for q in range(KLO):
        m = 1 << (N - 1 - q)
        sgn = np.where(idx & m, 1.0, -1.0)
        rows = np.concatenate([rows, sgn * rows[:, idx ^ m]], axis=0)

    # fold CNOT-chain permutation: final[j] = pre[g[j]]
    g = np.arange(DIM)[None, :]
    for c in range(N - 1):
        g = _apply_cnot_rows(g.astype(np.float64), c).astype(np.int64)
    rows = rows[:, g[0]]

    # relabel columns y = R x (y0=x0^x1, y1=x1^x2, y2=x2^x3, rest id):
    # in y-space each peeled rotation is a single bit-flip with sign = bit
    rows = rows[:, _x_of_y()]

    U = np.empty((NU, W2), dtype=np.float64)
    U[:, 0::2] = rows.real
    U[:, 1::2] = rows.imag
    return U


def _y_of_x():
    x = np.arange(DIM)
    x0, x1 = x & 1, (x >> 1) & 1
    x2, x3 = (x >> 2) & 1, (x >> 3) & 1
    return (x & ~7) | ((x2 ^ x3) << 2) | ((x1 ^ x2) << 1) | (x0 ^ x1)


def _x_of_y():
    y = _y_of_x()
    inv = np.empty(DIM, dtype=np.int64)
    inv[y] = np.arange(DIM)
    return inv


# ------------------------------------------------------------- bass kernel


def _rot_p2(eng, dst, src, u, s_pos, s_neg, block):
    """dst = c*src + (+-s)*src[col ^ block/2], sign +1 on the upper half of
    each block (u already holds c*src).  Columns are in y-space (linear
    relabeling chosen so each rotation is one bit-flip with sign = that
    bit), so both sub-calls are contiguous 3-dim APs."""
    H = block // 2
    vd = dst[:].rearrange("p (g u) -> p g u", u=block)
    vs = src[:].rearrange("p (g u) -> p g u", u=block)
    vu = u[:].rearrange("p (g u) -> p g u", u=block)
    eng.scalar_tensor_tensor(vd[:, :, H:], vs[:, :, :H], s_pos,
                             vu[:, :, H:], MUL, ADD)
    eng.scalar_tensor_tensor(vd[:, :, :H], vs[:, :, H:], s_neg,
                             vu[:, :, :H], MUL, ADD)


def build_kernel():
    nc = bacc.Bacc()
    x_d = nc.dram_tensor("x", (BSH, N), F32, kind="ExternalInput")
    u_d = nc.dram_tensor("u", (2, 128, W2), BF16, kind="ExternalInput")
    id_d = nc.dram_tensor("ident", (128, 128), BF16, kind="ExternalInput")
    out_d = nc.dram_tensor("out", (BSH, W2), BF16, kind="ExternalOutput")

    with tile.TileContext(nc) as tc:
        with (
            tc.tile_pool(name="const", bufs=1) as const_pool,
            tc.tile_pool(name="wbuild", bufs=2) as wbuild_pool,
            tc.tile_pool(name="wt", bufs=1) as wt_pool,
            tc.tile_pool(name="rot", bufs=2) as rot_pool,
            tc.tile_pool(name="ptr", bufs=2, space=bass.MemorySpace.PSUM) as ptr_pool,
            tc.tile_pool(name="pmm", bufs=4, space=bass.MemorySpace.PSUM) as pmm_pool,
        ):
            ident = const_pool.tile([128, 128], BF16)
            nc.gpsimd.dma_start(ident[:], id_d[:])

            # U replicated: 2 k-chunks of [128, 4096]
            u_sb = []
            for k in range(2):
                ut = const_pool.tile([128, W2], BF16, tag=f"u{k}")
                nc.sync.dma_start(ut[:], u_d[k])
                u_sb.append(ut)

            # x: (1024, 11) -> sbuf (128, 8*11); tile t in cols [t*11,(t+1)*11)
            x_sb = const_pool.tile([128, NTILES * N], F32)
            x_r = x_d.rearrange("(t p) f -> p t f", p=128)
            nc.gpsimd.dma_start(x_sb[:].rearrange("p (t f) -> p t f", f=N), x_r)

            cos_sb = const_pool.tile([128, NTILES * N], F32)
            sin_sb = const_pool.tile([128, NTILES * N], F32)
            nsin_sb = const_pool.tile([128, NTILES * N], F32)
            hp_t = const_pool.tile([128, 1], F32)
            zr_t = const_pool.tile([128, 1], F32)
            nc.vector.memset(hp_t[:], float(np.pi / 2))
            nc.vector.memset(zr_t[:], 0.0)
            # cos(t) = sin(pi/2 - t): keeps Sin args in (-pi/2, pi/2]
            nc.scalar.activation(cos_sb[:], x_sb[:],
                                 mybir.ActivationFunctionType.Sin,
                                 bias=hp_t[:], scale=-0.5)
            nc.scalar.activation(sin_sb[:], x_sb[:],
                                 mybir.ActivationFunctionType.Sin,
                                 bias=zr_t[:], scale=0.5)
            nc.scalar.activation(nsin_sb[:], x_sb[:],
                                 mybir.ActivationFunctionType.Sin,
                                 bias=zr_t[:], scale=-0.5)

            # Phase A: build transposed W_lo for every sample-tile
            wts = []
            for t in range(NTILES):
                col = t * N
                wa = wbuild_pool.tile([128, NU], F32, tag="wa")
                wb = wbuild_pool.tile([128, NU], F32, tag="wb")
                nc.vector.tensor_copy(wa[:, 0:1], cos_sb[:, col:col + 1])
                nc.vector.tensor_copy(wa[:, 1:2], sin_sb[:, col:col + 1])
                cur, nxt = wa, wb
                for j in range(1, KLO):
                    half = 1 << j
                    nc.vector.tensor_scalar_mul(
                        nxt[:, 0:half], cur[:, 0:half],
                        cos_sb[:, col + j:col + j + 1])
                    nc.vector.tensor_scalar_mul(
                        nxt[:, half:2 * half], cur[:, 0:half],
                        sin_sb[:, col + j:col + j + 1])
                    cur, nxt = nxt, cur
                wbf = wbuild_pool.tile([128, NU], BF16, tag="wbf")
                nc.vector.tensor_copy(wbf[:], cur[:])

                wt = wt_pool.tile([128, NU], BF16, tag=f"wt{t}")
                for k in range(2):
                    ptr = ptr_pool.tile([128, 128], BF16)
                    nc.tensor.transpose(ptr[:], wbf[:, k * 128:(k + 1) * 128],
                                        ident[:])
                    nc.vector.tensor_copy(wt[:, k * 128:(k + 1) * 128], ptr[:])
                wts.append(wt)

            # Phase B: matmul + rotations per sample-tile
            for t in range(NTILES):
                col = t * N
                S = rot_pool.tile([128, W2], BF16, tag="S")
                for ci in range(NCH):
                    pmm = pmm_pool.tile([128, CW], F32)
                    nc.tensor.matmul(pmm[:], wts[t][:, 0:128],
                                     u_sb[0][:, ci * CW:(ci + 1) * CW],
                                     start=True, stop=False)
                    nc.tensor.matmul(pmm[:], wts[t][:, 128:256],
                                     u_sb[1][:, ci * CW:(ci + 1) * CW],
                                     start=False, stop=True)
                    nc.scalar.copy(S[:, ci * CW:(ci + 1) * CW], pmm[:])

                V = rot_pool.tile([128, W2], BF16, tag="V")
                T1 = rot_pool.tile([128, W2], BF16, tag="T1")
                T2 = rot_pool.tile([128, W2], BF16, tag="T2")
                T3 = rot_pool.tile([128, W2], BF16, tag="T3")

                def csn(q):
                    return (cos_sb[:, col + q:col + q + 1],
                            sin_sb[:, col + q:col + q + 1],
                            nsin_sb[:, col + q:col + q + 1])

                c8, s8, n8 = csn(8)
                c9, s9, n9 = csn(9)
                c10, s10, n10 = csn(10)

                # r8: flip y-bit 2 -> col block 16
                nc.scalar.mul(V[:], S[:], c8)
                _rot_p2(nc.vector, T1, S, V, s8, n8, 16)
                # r9: flip y-bit 1 -> col block 8
                nc.vector.tensor_scalar_mul(V[:], T1[:], c9)
                _rot_p2(nc.vector, T2, T1, V, s9, n9, 8)
                # r10: flip y-bit 0 -> col block 4
                nc.vector.tensor_scalar_mul(V[:], T2[:], c10)
                _rot_p2(nc.vector, T3, T2, V, s10, n10, 4)

                nc.sync.dma_start(out_d[t * 128:(t + 1) * 128, :], T3[:])
    nc.finalize()
    return nc


# ----------------------------------------------------------------- driver

_CACHE = {}


def kernel(X, params):
    X = np.ascontiguousarray(np.asarray(X, dtype=np.float32))
    params = np.asarray(params, dtype=np.float32)

    U = build_u_matrix(params)
    u_bf = np.ascontiguousarray(
        U.reshape(2, 128, W2).astype(ml_dtypes.bfloat16))
    ident = np.eye(128, dtype=ml_dtypes.bfloat16)

    if "nc" not in _CACHE:
        _CACHE["nc"] = build_kernel()
    nc = _CACHE["nc"]

    in_maps = []
    for c in range(NCORES):
        in_maps.append({
            "x": X[c * BSH:(c + 1) * BSH],
            "u": u_bf,
            "ident": ident,
        })
    res = run_bass_kernel_spmd(nc, in_maps, list(range(NCORES)))
    out = np.concatenate([res.results[c]["out"] for c in range(NCORES)],
                         axis=0)
    # device columns are y-ordered; out[x] = dev[y(x)]
    out = out.astype(np.float32).reshape(BATCH, DIM, 2)
    return np.ascontiguousarray(out[:, _y_of_x(), :])


# revision 11
# speedup vs baseline: 5.2353x; 1.4020x over previous
"""Trainium2 Bass kernel for the DataReloadingQNN problem.

Math: layers 0..4 plus the shared RZ/RY/RZ of layer 5 collapse into one
fixed state w (params only).  The data gates are RY(x_q) = c_q I + s_q J_q
with J a signed permutation, all commuting.  Peel qubits 8,9,10: contract
the other eight via a dense matmul T = W_lo @ U with
    W_lo[b, m] = tensor product of 8 [cos,sin] pairs  (m in [0,256)),
    U[m, :]   = (P J^{(m)} w) re/im-interleaved, P = CNOT chain,
then apply per peeled qubit the per-sample rotation
    T <- c_q T + s_q * sign_q ( T[col ^ M_q] ).
Columns are relabeled host-side by a linear GF(2) map chosen so each
peeled rotation is a single y-bit flip with sign = that bit (bits 4,3,2
-> col blocks 64/32/16, contiguous halves).  The q=8 rotation is folded
into the matmul (second signed-permuted copy U8, weight variants c8*W
and s8*W -> K=512); q=9,10 run on VectorE as tensor_scalar (4x) +
tensor_tensor (2x) passes; ScalarE copies PSUM->SBUF, fusing the c9
scaling into one of the two copies.  Output is written bf16 in y-order;
the host upcasts and unpermutes.

Per core: 1024 samples = 8 tiles of 128.  Inputs sharded batch-wise
across 8 cores; U replicated.
"""
import numpy as np
import ml_dtypes

import concourse.bass as bass
import concourse.bacc as bacc
import concourse.tile as tile
from concourse import mybir
from concourse.bass_utils import run_bass_kernel_spmd

N = 11
DIM = 2048
BATCH = 8192
NCORES = 8
BSH = BATCH // NCORES          # 1024 samples per core
NTILES = BSH // 128            # 8 sample-tiles per core
KLO = 8                        # qubits contracted in the matmul
NU = 1 << KLO                  # 256 rows of U
W2 = 2 * DIM                   # 4096 output columns (re/im interleaved)
NCH = 8                        # 512-col chunks
CW = W2 // NCH
F32 = mybir.dt.float32
BF16 = mybir.dt.bfloat16

MUL = mybir.AluOpType.mult
ADD = mybir.AluOpType.add
SUB = mybir.AluOpType.subtract

# ---------------------------------------------------------------- host math


def _rz(phi):
    e = np.exp(-0.5j * phi)
    return np.array([[e, 0], [0, np.conj(e)]], dtype=np.complex128)


def _ry(theta):
    t = 0.5 * theta
    c, s = np.cos(t), np.sin(t)
    return np.array([[c, -s], [s, c]], dtype=np.complex128)


def _apply_1q_rows(rows, U, q):
    R = rows.shape[0]
    st = rows.reshape(R, 2 ** q, 2, 2 ** (N - 1 - q))
    st = np.einsum('ab,rxby->rxay', U, st)
    return st.reshape(R, DIM)


def _apply_cnot_rows(rows, c):
    R = rows.shape[0]
    st = rows.reshape(R, 2 ** c, 2, 2, 2 ** (N - 2 - c))
    st = np.stack([st[:, :, 0], st[:, :, 1, ::-1]], axis=2)
    return st.reshape(R, DIM)


def _y_of_x():
    """Column relabeling y = R x: y0=x3, y1=x4, y2=x0^x1, y3=x1^x2,
    y4=x2^x3, y5..10 = x5..x10 (bit i of the state index = 2^i)."""
    x = np.arange(DIM)
    x0, x1 = x & 1, (x >> 1) & 1
    x2, x3 = (x >> 2) & 1, (x >> 3) & 1
    x4 = (x >> 4) & 1
    return ((x & ~np.int64(31)) | (x3 << 0) | (x4 << 1)
            | ((x0 ^ x1) << 2) | ((x1 ^ x2) << 3) | ((x2 ^ x3) << 4))


def _x_of_y():
    y = _y_of_x()
    inv = np.empty(DIM, dtype=np.int64)
    inv[y] = np.arange(DIM)
    return inv


def build_u_matrices(params):
    """(6,11,3) f32 -> (Uy, U8), each (256, 4096) f64 in y-order.
    U8 is the signed bit-4-flip permutation of Uy (folds the q=8 gate)."""
    p = params.astype(np.float64)
    v = np.zeros((1, DIM), dtype=np.complex128)
    v[0, 0] = 1.0
    for l in range(5):
        for q in range(N):
            v = _apply_1q_rows(v, _rz(p[l, q, 0]), q)
            v = _apply_1q_rows(v, _ry(p[l, q, 1]), q)
            v = _apply_1q_rows(v, _rz(p[l, q, 2]), q)
        for c in range(N - 1):
            v = _apply_cnot_rows(v, c)
    for q in range(N):
        B = _rz(p[5, q, 2]) @ _ry(p[5, q, 1]) @ _rz(p[5, q, 0])
        v = _apply_1q_rows(v, B, q)

    # rows over J-subsets of qubits 0..7 (bit b of m <-> qubit b)
    rows = v
    idx = np.arange(DIM)
    for q in range(KLO):
        m = 1 << (N - 1 - q)
        sgn = np.where(idx & m, 1.0, -1.0)
        rows = np.concatenate([rows, sgn * rows[:, idx ^ m]], axis=0)

    # fold CNOT-chain permutation, then relabel columns to y-order
    g = np.arange(DIM)[None, :]
    for c in range(N - 1):
        g = _apply_cnot_rows(g.astype(np.float64), c).astype(np.int64)
    rows = rows[:, g[0]][:, _x_of_y()]

    # fold the q=8 rotation: U8 = sign(y bit 4) * Uy[:, y ^ 16]
    yy = np.arange(DIM)
    sgn8 = np.where((yy >> 4) & 1, 1.0, -1.0)
    rows8 = sgn8[None, :] * rows[:, yy ^ 16]

    def interleave(r):
        U = np.empty((NU, W2), dtype=np.float64)
        U[:, 0::2] = r.real
        U[:, 1::2] = r.imag
        return U

    return interleave(rows), interleave(rows8)


# ------------------------------------------------------------- bass kernel


def _rot_tt(nc, dst, u, w, block):
    """dst_hi = u_hi + w_lo ; dst_lo = u_lo - w_hi  per block."""
    H = block // 2
    vd = dst[:].rearrange("p (g u) -> p g u", u=block)
    vu = u[:].rearrange("p (g u) -> p g u", u=block)
    vw = w[:].rearrange("p (g u) -> p g u", u=block)
    nc.vector.tensor_tensor(vd[:, :, H:], vu[:, :, H:], vw[:, :, :H], ADD)
    nc.vector.tensor_tensor(vd[:, :, :H], vu[:, :, :H], vw[:, :, H:], SUB)


def build_kernel():
    nc = bacc.Bacc()
    x_d = nc.dram_tensor("x", (BSH, N), F32, kind="ExternalInput")
    u_d = nc.dram_tensor("u", (4, 128, W2), BF16, kind="ExternalInput")
    id_d = nc.dram_tensor("ident", (128, 128), BF16, kind="ExternalInput")
    out_d = nc.dram_tensor("out", (BSH, W2), BF16, kind="ExternalOutput")

    with tile.TileContext(nc) as tc:
        with (
            tc.tile_pool(name="const", bufs=1) as const_pool,
            tc.tile_pool(name="wbuild", bufs=2) as wbuild_pool,
            tc.tile_pool(name="wt", bufs=2) as wt_pool,
            tc.tile_pool(name="rot", bufs=2) as rot_pool,
            tc.tile_pool(name="ptr", bufs=2, space=bass.MemorySpace.PSUM) as ptr_pool,
            tc.tile_pool(name="pmm", bufs=1, space=bass.MemorySpace.PSUM) as pmm_pool,
        ):
            ident = const_pool.tile([128, 128], BF16)
            nc.gpsimd.dma_start(ident[:], id_d[:])

            # U replicated: Uy k0/k1, U8 k0/k1
            u_sb = []
            for k in range(4):
                ut = const_pool.tile([128, W2], BF16, tag=f"u{k}")
                nc.sync.dma_start(ut[:], u_d[k])
                u_sb.append(ut)

            # x: (1024, 11) -> sbuf (128, 8*11); tile t in cols [t*11,(t+1)*11)
            x_sb = const_pool.tile([128, NTILES * N], F32)
            x_r = x_d.rearrange("(t p) f -> p t f", p=128)
            nc.gpsimd.dma_start(x_sb[:].rearrange("p (t f) -> p t f", f=N), x_r)

            cos_sb = const_pool.tile([128, NTILES * N], F32)
            sin_sb = const_pool.tile([128, NTILES * N], F32)
            hp_t = const_pool.tile([128, 1], F32)
            zr_t = const_pool.tile([128, 1], F32)
            nc.vector.memset(hp_t[:], float(np.pi / 2))
            nc.vector.memset(zr_t[:], 0.0)
            # cos(t) = sin(pi/2 - t): keeps Sin args in (-pi/2, pi/2]
            nc.scalar.activation(cos_sb[:], x_sb[:],
                                 mybir.ActivationFunctionType.Sin,
                                 bias=hp_t[:], scale=-0.5)
            nc.scalar.activation(sin_sb[:], x_sb[:],
                                 mybir.ActivationFunctionType.Sin,
                                 bias=zr_t[:], scale=0.5)

            for t in range(NTILES):
                col = t * N

                def csn(q):
                    return (cos_sb[:, col + q:col + q + 1],
                            sin_sb[:, col + q:col + q + 1])

                c8, s8 = csn(8)
                c9, s9 = csn(9)
                c10, s10 = csn(10)

                # --- build W_lo (f32), scale into c8/s8 bf16 variants ---
                wa = wbuild_pool.tile([128, NU], F32, tag="wa")
                wb = wbuild_pool.tile([128, NU], F32, tag="wb")
                nc.vector.tensor_copy(wa[:, 0:1], cos_sb[:, col:col + 1])
                nc.vector.tensor_copy(wa[:, 1:2], sin_sb[:, col:col + 1])
                cur, nxt = wa, wb
                for j in range(1, KLO):
                    half = 1 << j
                    nc.vector.tensor_scalar_mul(
                        nxt[:, 0:half], cur[:, 0:half],
                        cos_sb[:, col + j:col + j + 1])
                    nc.vector.tensor_scalar_mul(
                        nxt[:, half:2 * half], cur[:, 0:half],
                        sin_sb[:, col + j:col + j + 1])
                    cur, nxt = nxt, cur
                wc = wbuild_pool.tile([128, NU], BF16, tag="wc")
                ws = wbuild_pool.tile([128, NU], BF16, tag="ws")
                nc.vector.tensor_scalar_mul(wc[:], cur[:], c8)
                nc.vector.tensor_scalar_mul(ws[:], cur[:], s8)

                wtc = wt_pool.tile([128, NU], BF16, tag="wtc")
                wts = wt_pool.tile([128, NU], BF16, tag="wts")
                for src, dst in ((wc, wtc), (ws, wts)):
                    for k in range(2):
                        ptr = ptr_pool.tile([128, 128], BF16)
                        nc.tensor.transpose(
                            ptr[:], src[:, k * 128:(k + 1) * 128], ident[:])
                        nc.vector.tensor_copy(
                            dst[:, k * 128:(k + 1) * 128], ptr[:])

                # --- matmuls: K=512 (c8*W @ Uy + s8*W @ U8), 2 chunk-groups
                # of 4 so weights are shared across 4 consecutive MMs ---
                S = rot_pool.tile([128, W2], BF16, tag="S")
                u9b = rot_pool.tile([128, W2], BF16, tag="u9b")
                pmms = []
                for g in range(2):
                    group = [pmm_pool.tile([128, CW], F32, tag=f"pmm{j}",
                                           name=f"pmm{j}")
                             for j in range(4)]
                    pmms.append(group)
                    for vi, (wv, ub) in enumerate(
                            ((wtc, 0), (wtc, 1), (wts, 2), (wts, 3))):
                        k = ub & 1
                        for cj in range(4):
                            ci = g * 4 + cj
                            nc.tensor.matmul(
                                group[cj][:],
                                wv[:, k * 128:(k + 1) * 128],
                                u_sb[ub][:, ci * CW:(ci + 1) * CW],
                                start=(vi == 0), stop=(vi == 3))
                    for cj in range(4):
                        ci = g * 4 + cj
                        sl = slice(ci * CW, (ci + 1) * CW)
                        nc.scalar.copy(S[:, sl], group[cj][:])
                        nc.scalar.mul(u9b[:, sl], group[cj][:], c9)

                # --- r9 (block 32): w9 = s9*S; T2 = u9b +- w9 halves ---
                w9 = rot_pool.tile([128, W2], BF16, tag="w9")
                T2 = rot_pool.tile([128, W2], BF16, tag="T2")
                nc.vector.tensor_scalar_mul(w9[:], S[:], s9)
                _rot_tt(nc, T2, u9b, w9, 32)

                # --- r10 (block 16) ---
                u10 = rot_pool.tile([128, W2], BF16, tag="u10")
                w10 = rot_pool.tile([128, W2], BF16, tag="w10")
                T3 = rot_pool.tile([128, W2], BF16, tag="T3")
                nc.vector.tensor_scalar_mul(u10[:], T2[:], c10)
                nc.vector.tensor_scalar_mul(w10[:], T2[:], s10)
                _rot_tt(nc, T3, u10, w10, 16)

                nc.sync.dma_start(out_d[t * 128:(t + 1) * 128, :], T3[:])
    nc.finalize()
    return nc


# ----------------------------------------------------------------- driver

_CACHE = {}


def kernel(X, params):
    X = np.ascontiguousarray(np.asarray(X, dtype=np.float32))
    params = np.asarray(params, dtype=np.float32)

    Uy, U8 = build_u_matrices(params)
    u_bf = np.ascontiguousarray(np.stack([
        Uy[:128], Uy[128:], U8[:128], U8[128:],
    ]).astype(ml_dtypes.bfloat16))
    ident = np.eye(128, dtype=ml_dtypes.bfloat16)

    if "nc" not in _CACHE:
        _CACHE["nc"] = build_kernel()
    nc = _CACHE["nc"]

    in_maps = []
    for c in range(NCORES):
        in_maps.append({
            "x": X[c * BSH:(c + 1) * BSH],
            "u": u_bf,
            "ident": ident,
        })
    res = run_bass_kernel_spmd(nc, in_maps, list(range(NCORES)))
    out = np.concatenate([res.results[c]["out"] for c in range(NCORES)],
                         axis=0)
    # device columns are y-ordered; out[x] = dev[y(x)]
    out = out.astype(np.float32).reshape(BATCH, DIM, 2)
    return np.ascontiguousarray(out[:, _y_of_x(), :])
